# revision 17
# baseline (speedup 1.0000x reference)
"""Trainium2 Bass kernel for the GaussianRenderer problem.

Contract: kernel(data, opacity) -> img
  data:    (32, 512, 8) float32
  opacity: (512, 1)     float32
  returns  (32, 3, 64, 64) float32

Sharding: data-parallel over batch B=32 across 8 NeuronCores (4 images
per core); no collectives.

Per-core algorithm (all compute on device):
  sigma[n, p] is a rank-6 bilinear form: sigma = F[n, :6] @ G[:6, p]
  where G rows are the pixel-coordinate monomials [1, x, y, x^2, y^2, xy]
  with x, y integer in [-32, 31] (exactly representable in fp16). F is
  derived per gaussian on-device; all transcendentals use only the
  exp_and_others ACT table set (tanh/abs/exp) -- the reference's
  sigmoid+sin are replaced by the identity sigmoid(x)-0.5 = tanh(x/2)/2
  plus Taylor polynomials for sin/cos(pi*v) and two double-angle steps
  on VectorE, so no ACT table reloads occur in steady state.

  F is split into fp16 hi/mid/lo parts and stacked K=18 so a single fp16
  TensorE matmul yields fp32-accurate (negated) sigma. The 16 gaussian
  tiles (4 images x 4 tiles of 128) are packed 4-per-image into the four
  32-row groups of the PE array (tile_position row tiling), so the four
  [18x128]@[18x512] sigma matmuls of one image run concurrently and fill
  four PSUM banks [128, 2048]; one ScalarE Exp instruction per 512-pixel
  segment evacuates all four banks to fp16 alpha in SBUF. Blending is a
  second TensorE matmul contracting the 128-gaussian partition dim with
  opacity-scaled fp16 colors (M=3), accumulated over the 4 gaussian
  tiles in PSUM, evacuated by a single VectorE copy, then DMA'd out.
"""

import numpy as np

import concourse.bacc as bacc
import concourse.mybir as mybir
import concourse.tile as tile
from concourse import bass_utils
from concourse._compat import get_trn_type
from concourse.alu_op_type import AluOpType

F32 = mybir.dt.float32
F16 = mybir.dt.float16
AF = mybir.ActivationFunctionType
MUL = AluOpType.mult
ADD = AluOpType.add
SUB = AluOpType.subtract

N_CORES = 8
B = 32
B_CORE = B // N_CORES  # 4 images per core
N = 512                # gaussians
NG = B_CORE * N        # gaussians handled per core
NT = 16                # gaussian tiles of 128 per core (4 img * 4 ntiles)
HW = 4096              # pixels per image (64 x 64)
PI = float(np.pi)


def host_constants():
    """G4 [128, 4096] fp16: partition 32*g + j (g=0..3, j=0..17) holds
    monomial row j%6 of [1, x, y, x^2, y^2, xy] (hi/mid/lo K-stacking of
    3 copies of the 6 rows), with x, y integer in [-32, 31]; partitions
    j>=18 are zero.  Plus fp16 identity for the PE transpose."""
    xs = np.arange(64, dtype=np.float64) - 32.0
    Xg, Yg = np.meshgrid(xs, xs)  # [h, w]; row-major pixels p = h*64 + w
    G = np.stack(
        [np.ones_like(Xg), Xg, Yg, Xg * Xg, Yg * Yg, Xg * Yg], 0
    ).reshape(6, HW)
    G4 = np.zeros((128, HW), np.float16)
    for g in range(4):
        for j in range(18):
            G4[32 * g + j] = G[j % 6]
    ident = np.eye(128, dtype=np.float16)
    return G4, ident


def build_program(reps=1, loop=0, skip_prep=False, skip_sigma=False,
                  skip_exp=False, skip_blend=False, seg_major=True,
                  split_exp=1, sig_bufs=3, sig_cols=1024, blend_bufs=2,
                  min_sigma=False):
    import contextlib

    nc = bacc.Bacc(get_trn_type() or "TRN2", target_bir_lowering=False, debug=False)
    # host pre-permutes into the on-chip layouts so these DMAs are contiguous
    d_data = nc.dram_tensor("data", (128, 128), F32, kind="ExternalInput")
    d_opac = nc.dram_tensor("opacity", (128, 4), F32, kind="ExternalInput")
    d_g4 = nc.dram_tensor("gconst", (128, HW), F16, kind="ExternalInput")
    d_id = nc.dram_tensor("ident", (128, 128), F16, kind="ExternalInput")
    d_img = nc.dram_tensor("img", (B_CORE, 3, 64, 64), F32, kind="ExternalOutput")

    with tile.TileContext(nc) as tc:
      _loop_kw = dict(
          hint_engines=(
              mybir.EngineType.PE,
              mybir.EngineType.Activation,
              mybir.EngineType.DVE,
              mybir.EngineType.SP,
              mybir.EngineType.Pool,
          )
      )
      with tc.For_i(0, loop, 1, **_loop_kw) if loop else contextlib.nullcontext():
       for rep in range(reps):
        _r = f"r{rep}_" if reps > 1 else ""
        with (
            tc.tile_pool(name=_r + "const", bufs=1) as constp,
            tc.tile_pool(name=_r + "prep", bufs=1) as prep,
            tc.tile_pool(name=_r + "alpha", bufs=2) as alphap,
            tc.tile_pool(name=_r + "outp", bufs=4) as outp,
        ):
            # ---- constants + inputs to SBUF ----
            g4 = constp.tile([128, HW], F16, tag="g4")
            nc.sync.dma_start(g4[:], d_g4[:])
            idt = constp.tile([128, 128], F16, tag="idt")
            nc.sync.dma_start(idt[:], d_id[:])
            d8 = constp.tile([128, 128], F32, tag="d8")  # [p, t*8+k]
            nc.sync.dma_start(d8[:], d_data[:])
            opac = constp.tile([128, 4], F32, tag="opac")  # [p, ntile]
            nc.sync.dma_start(opac[:], d_opac[:])

            f4 = constp.tile([128, B_CORE * 128], F16, tag="f4")
            c2 = constp.tile([128, NT * 3], F16, tag="c2")

            d8v = d8.rearrange("p (t k) -> p t k", k=8)

            def field(k):  # [128, 16] strided view of input field k
                return d8v[:, :, k]

            def t16(tag):
                return prep.tile([128, 16], F32, tag=tag, name=_r + tag)

            def _do_prep():
                # ---- per-gaussian preprocessing ([128, 16] fp32 tiles) ----
                # theta = 2*pi*sigmoid(d4); only cos/sin(2*theta) are needed.
                # v = sigmoid(d4) - 0.5 = tanh(d4/2)/2, so with p = pi*v:
                # cos(2t) = cos(4*pi*v), sin(2t) = sin(4*pi*v), built from
                # Taylor cos/sin(p) (|p| < pi/2) + two double-angle steps.
                # Everything stays in the exp_and_others ACT table set.
                th = t16("th")
                nc.scalar.activation(th[:], field(4), AF.Tanh, scale=0.5)
                pv = t16("pv")
                nc.vector.tensor_scalar_mul(pv[:], th[:], PI / 2)
                wv = t16("wv")
                nc.scalar.activation(wv[:], pv[:], AF.Square)

                def horner(dst, coeffs, tail):
                    # dst = (((c0*w + c1)*w + c2)...)*w + tail
                    nc.vector.tensor_scalar(
                        dst[:], wv[:], coeffs[0], coeffs[1], MUL, ADD
                    )
                    for ck in coeffs[2:] + [tail]:
                        nc.vector.tensor_tensor(dst[:], dst[:], wv[:], MUL)
                        nc.vector.tensor_scalar_add(dst[:], dst[:], ck)

                cw = t16("cw")  # cos(p)
                horner(
                    cw,
                    [-1.0 / 3628800, 1.0 / 40320, -1.0 / 720, 1.0 / 24, -0.5],
                    1.0,
                )
                sw = t16("sw")  # sin(p) = p * poly(w)
                horner(sw, [1.0 / 362880, -1.0 / 5040, 1.0 / 120, -1.0 / 6], 1.0)
                nc.vector.tensor_tensor(sw[:], sw[:], pv[:], MUL)

                c1t = t16("c1t")  # cos(2p)
                nc.vector.tensor_tensor(c1t[:], sw[:], sw[:], MUL)
                nc.vector.tensor_scalar(c1t[:], c1t[:], -2.0, 1.0, MUL, ADD)
                s1t = t16("s1t")  # sin(2p)
                nc.vector.scalar_tensor_tensor(s1t[:], sw[:], 2.0, cw[:], MUL, MUL)
                c2t = t16("c2t")  # cos(4p) = cos(2*theta)
                nc.vector.tensor_tensor(c2t[:], s1t[:], s1t[:], MUL)
                nc.vector.tensor_scalar(c2t[:], c2t[:], -2.0, 1.0, MUL, ADD)
                s2t = t16("s2t")  # sin(4p) = sin(2*theta)
                nc.vector.scalar_tensor_tensor(s2t[:], s1t[:], 2.0, c1t[:], MUL, MUL)

                # centers (fields 0,1 fused): exy = 32*tanh(d01) - 0.5
                exy = prep.tile([128, 32], F32, tag="exy", name=_r + "exy")
                exy3 = exy.rearrange("p (t k) -> p t k", k=2)
                nc.scalar.activation(exy3[:, :, :], d8v[:, :, 0:2], AF.Tanh)
                nc.vector.tensor_scalar(exy[:], exy[:], 32.0, -0.5, MUL, ADD)
                ex = exy3[:, :, 0]
                ey = exy3[:, :, 1]

                # scales (fields 2,3 fused): h01 = 0.5*(|d23|+0.3)^2
                s01 = prep.tile([128, 32], F32, tag="s01", name=_r + "s01")
                s013 = s01.rearrange("p (t k) -> p t k", k=2)
                nc.scalar.activation(s013[:, :, :], d8v[:, :, 2:4], AF.Abs)
                nc.vector.tensor_scalar_add(s01[:], s01[:], 0.3)
                h01 = prep.tile([128, 32], F32, tag="h01", name=_r + "h01")
                nc.scalar.activation(
                    h01[:], s01[:], AF.Square, scale=float(np.sqrt(0.5))
                )
                h013 = h01.rearrange("p (t k) -> p t k", k=2)

                sum5 = t16("sum5")  # 0.5*(s0^2+s1^2)
                nc.vector.tensor_tensor(sum5[:], h013[:, :, 0], h013[:, :, 1], ADD)
                dif5 = t16("dif5")  # 0.5*(s0^2-s1^2)
                nc.vector.tensor_tensor(dif5[:], h013[:, :, 0], h013[:, :, 1], SUB)

                # covariance entries
                dc = t16("dc")
                nc.vector.tensor_tensor(dc[:], dif5[:], c2t[:], MUL)
                cov_a = t16("cov_a")
                nc.vector.tensor_tensor(cov_a[:], sum5[:], dc[:], ADD)
                cov_c = t16("cov_c")
                nc.vector.tensor_tensor(cov_c[:], sum5[:], dc[:], SUB)
                cov_b = t16("cov_b")
                nc.vector.tensor_tensor(cov_b[:], dif5[:], s2t[:], MUL)

                det = t16("det")
                nc.vector.tensor_tensor(det[:], cov_a[:], cov_c[:], MUL)
                bb = t16("bb")
                nc.scalar.activation(bb[:], cov_b[:], AF.Square)
                nc.vector.tensor_tensor(det[:], det[:], bb[:], SUB)

                # conic: ca = cov_c/det, cc = cov_a/det, cbn = cov_b/det
                inv = t16("inv")
                nc.vector.reciprocal(inv[:], det[:])
                ca = t16("ca")
                nc.vector.tensor_tensor(ca[:], cov_c[:], inv[:], MUL)
                cc = t16("cc")
                nc.vector.tensor_tensor(cc[:], cov_a[:], inv[:], MUL)
                cbn = t16("cbn")
                nc.vector.tensor_tensor(cbn[:], cov_b[:], inv[:], MUL)

                # ---- F rows (negated for exp), written into Fc [128, 96] ----
                Fc = prep.tile([128, 96], F32, tag="Fc")
                Fv = Fc.rearrange("p (t k) -> p t k", k=6)

                exyq = prep.tile([128, 32], F32, tag="exyq", name=_r + "exyq")
                nc.scalar.activation(exyq[:], exy[:], AF.Square)
                exyq3 = exyq.rearrange("p (t k) -> p t k", k=2)
                exq = exyq3[:, :, 0]
                eyq = exyq3[:, :, 1]
                exey = t16("exey")
                nc.vector.tensor_tensor(exey[:], ex, ey, MUL)

                # f0 = -0.5*ca*exq - 0.5*cc*eyq + cbn*exey
                t_a = t16("t_a")
                nc.vector.tensor_tensor(t_a[:], ca[:], exq, MUL)
                t_b = t16("t_b")
                nc.vector.tensor_tensor(t_b[:], cc[:], eyq, MUL)
                nc.vector.tensor_tensor(t_a[:], t_a[:], t_b[:], ADD)
                nc.vector.tensor_scalar_mul(t_a[:], t_a[:], -0.5)
                t_c = t16("t_c")
                nc.vector.tensor_tensor(t_c[:], cbn[:], exey[:], MUL)
                nc.vector.tensor_tensor(Fv[:, :, 0], t_a[:], t_c[:], ADD)

                # f_x = ca*ex - cbn*ey ; f_y = cc*ey - cbn*ex
                nc.vector.tensor_tensor(t_a[:], ca[:], ex, MUL)
                nc.vector.tensor_tensor(t_b[:], cbn[:], ey, MUL)
                nc.vector.tensor_tensor(Fv[:, :, 1], t_a[:], t_b[:], SUB)
                nc.vector.tensor_tensor(t_a[:], cc[:], ey, MUL)
                nc.vector.tensor_tensor(t_b[:], cbn[:], ex, MUL)
                nc.vector.tensor_tensor(Fv[:, :, 2], t_a[:], t_b[:], SUB)

                # f_x2 = -0.5*ca ; f_y2 = -0.5*cc ; f_xy = +cbn
                nc.vector.tensor_scalar_mul(Fv[:, :, 3], ca[:], -0.5)
                nc.vector.tensor_scalar_mul(Fv[:, :, 4], cc[:], -0.5)
                nc.vector.tensor_scalar_mul(Fv[:, :, 5], cbn[:], 1.0)

                # ---- split F into fp16 hi/mid/lo, [128, 16*32] (stride 32
                # so one full [128,128] transpose per image lands each tile
                # at the 32-row group the sigma matmul reads) ----
                fall = prep.tile([128, NT * 32], F16, tag="fall")
                nc.vector.memset(fall[:], 0.0)
                fv = fall.rearrange("p (t s) -> p t s", s=32)
                Fc6 = Fc.rearrange("p (t k) -> p t k", k=6)
                nc.vector.tensor_copy(fv[:, :, 0:6], Fc6[:, :, :])
                r1 = prep.tile([128, 96], F32, tag="r1")
                r16 = r1.rearrange("p (t k) -> p t k", k=6)
                nc.vector.tensor_tensor(
                    r16[:, :, :], Fc6[:, :, :], fv[:, :, 0:6], SUB
                )
                nc.vector.tensor_copy(fv[:, :, 6:12], r16[:, :, :])
                r2 = prep.tile([128, 96], F32, tag="r2")
                r26 = r2.rearrange("p (t k) -> p t k", k=6)
                nc.vector.tensor_tensor(
                    r26[:, :, :], r16[:, :, :], fv[:, :, 6:12], SUB
                )
                nc.vector.tensor_copy(fv[:, :, 12:18], r26[:, :, :])

                # ---- per-image transpose: [128, 128] -> psum -> f4 ----
                with tc.tile_pool(name=_r + "prepps", bufs=2, space="PSUM") as prepps:
                    for img in range(B_CORE):
                        tp = prepps.tile(
                            [128, 128], F16, tag="tp", name=f"{_r}tp{img}"
                        )
                        nc.tensor.transpose(
                            tp[:], fall[:, img * 128 : (img + 1) * 128], idt[:]
                        )
                        nc.vector.tensor_copy(
                            f4[:, img * 128 : (img + 1) * 128], tp[:]
                        )

                # ---- colors * opacity -> c2 [128, 16*3] fp16 ----
                opac_b = opac[:].unsqueeze(1).broadcast_to([128, 4, 4])
                cP = prep.tile([128, 48], F32, tag="cP")
                cP4 = cP.rearrange("p (i n k) -> p i n k", n=4, k=3)
                d84 = d8.rearrange("p (i n k) -> p i n k", n=4, k=8)
                for k in range(3):
                    nc.vector.tensor_tensor(
                        cP4[:, :, :, k], d84[:, :, :, 5 + k], opac_b, MUL
                    )
                nc.vector.tensor_copy(c2[:], cP[:])

            if not skip_prep:
                _do_prep()

            # ---- main loop ----
            # alpha layout: seg-major [128, seg*2048 + nt*512 + q] so the
            # Exp write is fully contiguous and the blend rhs slices are
            # contiguous too (blend chunk ch == pixel segment seg).
            def al_exp_view(al, seg):
                if seg_major:
                    return al[:, seg * 2048 : (seg + 1) * 2048]
                return al.rearrange("p (t q) -> p t q", q=HW)[
                    :, :, seg * 512 : (seg + 1) * 512
                ]

            def al_blend_view(al, ch, nt):
                if seg_major:
                    off = ch * 2048 + nt * 512
                else:
                    off = nt * HW + ch * 512
                return al[:, off : off + 512]

            n_grp = sig_cols // 512  # row groups covered per PSUM tile
            with (
                tc.tile_pool(name=_r + "sigps", bufs=sig_bufs, space="PSUM") as sigps,
                tc.tile_pool(name=_r + "blps", bufs=blend_bufs, space="PSUM") as blps,
            ):
                als = {}

                def emit_sigma(img, seg):
                    al = als[img]
                    for h in range(4 // n_grp):
                        sps = sigps.tile(
                            [128, sig_cols], F32, tag="sig",
                            name=f"{_r}sig{img}_{seg}_{h}"
                        )
                        if not skip_sigma:
                            nts = [0] if min_sigma else range(n_grp)
                            for j in nts:
                                nt = h * n_grp + j
                                nc.tensor.matmul(
                                    sps[:, j * 512 : (j + 1) * 512],
                                    f4[32 * nt : 32 * nt + 18,
                                       img * 128 : (img + 1) * 128],
                                    g4[32 * nt : 32 * nt + 18,
                                       seg * 512 : (seg + 1) * 512],
                                    start=True,
                                    stop=True,
                                    tile_position=(32 * nt, 0),
                                )
                        if not skip_exp:
                            step = sig_cols // split_exp
                            base = h * sig_cols
                            for q in range(0, sig_cols, step):
                                nc.scalar.activation(
                                    al_exp_view(als[img], seg)[
                                        :, base + q : base + q + step
                                    ],
                                    sps[:, q : q + step],
                                    AF.Exp,
                                )

                def emit_blend(img, ch):
                    al = als[img]
                    bps = blps.tile(
                        [3, 512], F32, tag="bl", name=f"{_r}bl{img}_{ch}"
                    )
                    for nt in range(4):
                        t = img * 4 + nt
                        nc.tensor.matmul(
                            bps[:],
                            c2[:, t * 3 : t * 3 + 3],
                            (g4[:, ch * 512 : ch * 512 + 512] if skip_exp
                             else al_blend_view(al, ch, nt)),
                            start=(nt == 0),
                            stop=(nt == 3),
                        )
                    ot = outp.tile([3, 512], F32, tag="ot", name=f"{_r}ot{img}_{ch}")
                    nc.vector.tensor_copy(ot[:], bps[:])
                    nc.sync.dma_start(
                        d_img[img, :, ch * 8 : (ch + 1) * 8, :].rearrange(
                            "c h w -> c (h w)"
                        ),
                        ot[:],
                    )

                # software pipeline: interleave img i's sigma/exp segments
                # with img i-1's blend chunks so PE never parks a whole
                # blend pass between ACT's exp streams.
                for img in range(B_CORE + 1):
                    if img < B_CORE:
                        als[img] = alphap.tile(
                            [128, 4 * HW], F16, tag="al", name=f"{_r}al{img}"
                        )
                    for seg in range(8):
                        if img < B_CORE:
                            emit_sigma(img, seg)
                        if img >= 1 and not skip_blend:
                            emit_blend(img - 1, seg)

    nc.compile()
    return nc


_NC_CACHE = None


def _get_program():
    global _NC_CACHE
    if _NC_CACHE is None:
        _NC_CACHE = build_program()
    return _NC_CACHE


def make_in_maps(data, opacity):
    data = np.ascontiguousarray(np.asarray(data, dtype=np.float32))
    opacity = np.ascontiguousarray(np.asarray(opacity, dtype=np.float32))
    G4, ident = host_constants()
    in_maps = []
    op_pt = np.ascontiguousarray(opacity.reshape(4, 128).T)  # [p, ntile]
    for c in range(N_CORES):
        dc = data[c * B_CORE : (c + 1) * B_CORE].reshape(NG, 8)
        # device layout d8[p, t*8+k] = data[t*128+p, k]
        d8 = np.ascontiguousarray(
            dc.reshape(NT, 128, 8).transpose(1, 0, 2).reshape(128, 128)
        )
        in_maps.append(
            {"data": d8, "opacity": op_pt, "gconst": G4, "ident": ident}
        )
    return in_maps


class _Executor:
    """One-time jit of the sharded bass_exec program + device-resident
    constants; warm kernel() calls only upload data/opacity and download
    the image, avoiding the per-call re-trace/re-compile/NEFF-reload that
    run_bass_kernel_spmd pays."""

    def __init__(self):
        import jax
        from jax.sharding import Mesh, NamedSharding, PartitionSpec
        from jax.experimental.shard_map import shard_map
        from concourse import bass2jax

        bass2jax.install_neuronx_cc_hook()
        nc = _get_program()
        self.jax = jax
        partition_name = (
            nc.partition_id_tensor.name if nc.partition_id_tensor else None
        )
        in_names, out_names, out_avals, zero_outs = [], [], [], []
        for alloc in nc.m.functions[0].allocations:
            if not isinstance(alloc, mybir.MemoryLocationSet):
                continue
            name = alloc.memorylocations[0].name
            if alloc.kind == "ExternalInput":
                if name != partition_name:
                    in_names.append(name)
            elif alloc.kind == "ExternalOutput":
                out_names.append(name)
                shape = tuple(alloc.tensor_shape)
                dtype = mybir.dt.np(alloc.dtype)
                out_avals.append(jax.core.ShapedArray(shape, dtype))
                zero_outs.append(np.zeros(shape, dtype))
        n_params = len(in_names)
        n_outs = len(out_avals)
        all_in_names = list(in_names) + out_names
        if partition_name is not None:
            all_in_names.append(partition_name)

        def _body(*args):
            operands = list(args)
            if partition_name is not None:
                operands.append(bass2jax.partition_id_tensor())
            outs = bass2jax._bass_exec_p.bind(
                *operands,
                out_avals=tuple(out_avals),
                in_names=tuple(all_in_names),
                out_names=tuple(out_names),
                lowering_input_output_aliases=(),
                sim_require_finite=True,
                sim_require_nnan=True,
                nc=nc,
            )
            return tuple(outs)

        devices = jax.devices()[:N_CORES]
        mesh = Mesh(np.asarray(devices), ("core",))
        in_specs = (PartitionSpec("core"),) * (n_params + n_outs)
        out_specs = (PartitionSpec("core"),) * n_outs
        donate = tuple(range(n_params, n_params + n_outs))
        self.sharded = jax.jit(
            shard_map(
                _body,
                mesh=mesh,
                in_specs=in_specs,
                out_specs=out_specs,
                check_rep=False,
            ),
            donate_argnums=donate,
            keep_unused=True,
        )
        self.in_names = in_names
        self.out_names = out_names
        self.out_avals = out_avals
        sh = NamedSharding(mesh, PartitionSpec("core"))
        self.sharding = sh
        # constants resident on device, sharded by core
        G4, ident = host_constants()
        self.const_dev = {
            "gconst": jax.device_put(
                np.concatenate([G4] * N_CORES, axis=0), sh
            ),
            "ident": jax.device_put(
                np.concatenate([ident] * N_CORES, axis=0), sh
            ),
        }
        # donated output buffers: zeros on first call, recycled afterwards
        # (the kernel writes every output element)
        self.out_bufs = [
            jax.device_put(
                np.zeros((N_CORES * z.shape[0], *z.shape[1:]), z.dtype), sh
            )
            for z in zero_outs
        ]

    def run(self, per_core_inputs):
        """per_core_inputs: dict name -> list of per-core np arrays (for
        non-constant inputs)."""
        args = []
        for name in self.in_names:
            if name in self.const_dev:
                args.append(self.const_dev[name])
            else:
                args.append(np.concatenate(per_core_inputs[name], axis=0))
        outs = self.sharded(*args, *self.out_bufs)
        self.out_bufs = list(outs)
        return outs


_EXEC_CACHE = None


def _get_executor():
    global _EXEC_CACHE
    if _EXEC_CACHE is None:
        _EXEC_CACHE = _Executor()
    return _EXEC_CACHE


def kernel(data, opacity):
    ex = _get_executor()
    data = np.ascontiguousarray(np.asarray(data, dtype=np.float32))
    opacity = np.ascontiguousarray(np.asarray(opacity, dtype=np.float32))
    op_pt = np.ascontiguousarray(opacity.reshape(4, 128).T)
    d8s = [
        np.ascontiguousarray(
            data[c * B_CORE : (c + 1) * B_CORE]
            .reshape(NT, 128, 8)
            .transpose(1, 0, 2)
            .reshape(128, 128)
        )
        for c in range(N_CORES)
    ]
    outs = ex.run({"data": d8s, "opacity": [op_pt] * N_CORES})
    i_img = ex.out_names.index("img")
    img = np.asarray(outs[i_img]).reshape(N_CORES, B_CORE, 3, 64, 64)
    return img.reshape(B, 3, 64, 64).astype(np.float32)


# revision 32
# speedup vs baseline: 1.4579x; 1.4579x over previous
"""Trainium2 Bass kernel for the GaussianRenderer problem.

Contract: kernel(data, opacity) -> img
  data:    (32, 512, 8) float32
  opacity: (512, 1)     float32
  returns  (32, 3, 64, 64) float32

Sharding: data-parallel over batch B=32 across 8 NeuronCores (4 images
per core); no collectives.

Per-core algorithm (all compute on device):
  sigma[n, p] is a rank-6 bilinear form: sigma = F[n, :6] @ G[:6, p]
  where G rows are the pixel-coordinate monomials [1, x, y, x^2, y^2, xy]
  with x, y integer in [-32, 31] (exactly representable in fp16). F is
  derived per gaussian on-device; all transcendentals use only the
  exp_and_others ACT table set (tanh/abs/exp) -- the reference's
  sigmoid+sin are replaced by the identity sigmoid(x)-0.5 = tanh(x/2)/2
  plus Taylor polynomials for sin/cos(pi*v) and two double-angle steps
  on VectorE, so no ACT table reloads occur in steady state.

  F is split into fp16 hi/mid/lo parts and stacked K=18 so a single fp16
  TensorE matmul yields fp32-accurate (negated) sigma. The 16 gaussian
  tiles (4 images x 4 tiles of 128) are packed 4-per-image into the four
  32-row groups of the PE array (tile_position row tiling), so the four
  [18x128]@[18x512] sigma matmuls of one image run concurrently and fill
  four PSUM banks [128, 2048]; one ScalarE Exp instruction per 512-pixel
  segment evacuates all four banks to fp16 alpha in SBUF. Blending is a
  second TensorE matmul contracting the 128-gaussian partition dim with
  opacity-scaled fp16 colors (M=3), accumulated over the 4 gaussian
  tiles in PSUM, evacuated by a single VectorE copy, then DMA'd out.
"""

import numpy as np

import concourse.bacc as bacc
import concourse.mybir as mybir
import concourse.tile as tile
from concourse import bass_utils
from concourse._compat import get_trn_type
from concourse.alu_op_type import AluOpType

F32 = mybir.dt.float32
F16 = mybir.dt.float16
AF = mybir.ActivationFunctionType
MUL = AluOpType.mult
ADD = AluOpType.add
SUB = AluOpType.subtract

N_CORES = 8
B = 32
B_CORE = B // N_CORES  # 4 images per core
N = 512                # gaussians
NG = B_CORE * N        # gaussians handled per core
NT = 16                # gaussian tiles of 128 per core (4 img * 4 ntiles)
HW = 4096              # pixels per image (64 x 64)
PI = float(np.pi)


def host_constants():
    """G4 [128, 4096] fp16: partition 32*g + j (g=0..3, j=0..17) holds
    monomial row j%6 of [1, x, y, x^2, y^2, xy] (hi/mid/lo K-stacking of
    3 copies of the 6 rows), with x, y integer in [-32, 31]; partitions
    j>=18 are zero.  Plus fp16 identity for the PE transpose."""
    xs = np.arange(64, dtype=np.float64) - 32.0
    Xg, Yg = np.meshgrid(xs, xs)  # [h, w]; row-major pixels p = h*64 + w
    G = np.stack(
        [np.ones_like(Xg), Xg, Yg, Xg * Xg, Yg * Yg, Xg * Yg], 0
    ).reshape(6, HW)
    G18 = np.concatenate([G, G, G], 0).astype(np.float16)  # [18, HW]
    ident = np.eye(128, dtype=np.float16)
    return G18, ident


def build_program(reps=1, loop=0, skip_prep=False, skip_sigma=False,
                  skip_exp=False, skip_blend=False, seg_major=True,
                  split_exp=1, sig_bufs=2, sig_cols=1024, blend_bufs=1,
                  blend_lag=1, blend_skew=0, blend_groups=4, min_sigma=False,
                  prep_gpsimd=False):
    import contextlib

    nc = bacc.Bacc(get_trn_type() or "TRN2", target_bir_lowering=False, debug=False)
    # host pre-permutes into the on-chip layouts so these DMAs are contiguous
    d_data = nc.dram_tensor("data", (128, 128), F32, kind="ExternalInput")
    d_opac = nc.dram_tensor("opacity", (128, 4), F32, kind="ExternalInput")
    d_g4 = nc.dram_tensor("gconst", (18, HW), F16, kind="ExternalInput")
    d_id = nc.dram_tensor("ident", (128, 128), F16, kind="ExternalInput")
    d_img = nc.dram_tensor("img", (B_CORE, 3, 64, 64), F32, kind="ExternalOutput")

    with tile.TileContext(nc) as tc:
      _loop_kw = dict(
          hint_engines=(
              mybir.EngineType.PE,
              mybir.EngineType.Activation,
              mybir.EngineType.DVE,
              mybir.EngineType.SP,
              mybir.EngineType.Pool,
          )
      )
      with tc.For_i(0, loop, 1, **_loop_kw) if loop else contextlib.nullcontext():
       for rep in range(reps):
        _r = f"r{rep}_" if reps > 1 else ""
        with (
            tc.tile_pool(name=_r + "const", bufs=1) as constp,
            tc.tile_pool(name=_r + "prep", bufs=1) as prep,
            tc.tile_pool(name=_r + "alpha", bufs=blend_lag + 2) as alphap,
            tc.tile_pool(name=_r + "outp", bufs=4) as outp,
        ):
            # ---- constants + inputs to SBUF ----
            # monomials replicated into the four 32-row groups (rows 18-31
            # of each group are never read)
            g4 = constp.tile([128, HW], F16, tag="g4")
            for g in range(4):
                nc.sync.dma_start(g4[32 * g : 32 * g + 18, :], d_g4[:])
            idt = constp.tile([128, 128], F16, tag="idt")
            nc.sync.dma_start(idt[:], d_id[:])
            d8 = constp.tile([128, 128], F32, tag="d8")  # [p, t*8+k]
            nc.sync.dma_start(d8[:], d_data[:])
            opac = constp.tile([128, 4], F32, tag="opac")  # [p, ntile]
            nc.sync.dma_start(opac[:], d_opac[:])

            f4 = constp.tile([128, B_CORE * 128], F16, tag="f4")
            c2 = constp.tile([128, NT * 3], F16, tag="c2")

            d8v = d8.rearrange("p (t k) -> p t k", k=8)

            def field(k):  # [128, 16] strided view of input field k
                return d8v[:, :, k]

            def t16(tag):
                return prep.tile([128, 16], F32, tag=tag, name=_r + tag)

            def _do_prep():
                # ---- per-gaussian preprocessing ([128, 16] fp32 tiles) ----
                # theta = 2*pi*sigmoid(d4); only cos/sin(2*theta) are needed.
                # v = sigmoid(d4) - 0.5 = tanh(d4/2)/2, so with p = pi*v:
                # cos(2t) = cos(4*pi*v), sin(2t) = sin(4*pi*v), built from
                # Taylor cos/sin(p) (|p| < pi/2) + two double-angle steps.
                # Everything stays in the exp_and_others ACT table set.
                th = t16("th")
                nc.scalar.activation(th[:], field(4), AF.Tanh, scale=0.5)
                pv = t16("pv")
                nc.vector.tensor_scalar_mul(pv[:], th[:], PI / 2)
                wv = t16("wv")
                nc.scalar.activation(wv[:], pv[:], AF.Square)

                def horner(dst, coeffs, tail):
                    # dst = (((c0*w + c1)*w + c2)...)*w + tail
                    nc.vector.tensor_scalar(
                        dst[:], wv[:], coeffs[0], coeffs[1], MUL, ADD
                    )
                    for ck in coeffs[2:] + [tail]:
                        nc.vector.tensor_tensor(dst[:], dst[:], wv[:], MUL)
                        nc.vector.tensor_scalar_add(dst[:], dst[:], ck)

                cw = t16("cw")  # cos(p)
                horner(
                    cw,
                    [-1.0 / 3628800, 1.0 / 40320, -1.0 / 720, 1.0 / 24, -0.5],
                    1.0,
                )
                sw = t16("sw")  # sin(p) = p * poly(w)
                horner(sw, [1.0 / 362880, -1.0 / 5040, 1.0 / 120, -1.0 / 6], 1.0)
                nc.vector.tensor_tensor(sw[:], sw[:], pv[:], MUL)

                c1t = t16("c1t")  # cos(2p)
                nc.vector.tensor_tensor(c1t[:], sw[:], sw[:], MUL)
                nc.vector.tensor_scalar(c1t[:], c1t[:], -2.0, 1.0, MUL, ADD)
                s1t = t16("s1t")  # sin(2p)
                nc.vector.scalar_tensor_tensor(s1t[:], sw[:], 2.0, cw[:], MUL, MUL)
                c2t = t16("c2t")  # cos(4p) = cos(2*theta)
                nc.vector.tensor_tensor(c2t[:], s1t[:], s1t[:], MUL)
                nc.vector.tensor_scalar(c2t[:], c2t[:], -2.0, 1.0, MUL, ADD)
                s2t = t16("s2t")  # sin(4p) = sin(2*theta)
                nc.vector.scalar_tensor_tensor(s2t[:], s1t[:], 2.0, c1t[:], MUL, MUL)

                # centers (fields 0,1 fused): exy = 32*tanh(d01) - 0.5
                exy = prep.tile([128, 32], F32, tag="exy", name=_r + "exy")
                exy3 = exy.rearrange("p (t k) -> p t k", k=2)
                nc.scalar.activation(exy3[:, :, :], d8v[:, :, 0:2], AF.Tanh)
                nc.vector.tensor_scalar(exy[:], exy[:], 32.0, -0.5, MUL, ADD)
                ex = exy3[:, :, 0]
                ey = exy3[:, :, 1]

                # scales (fields 2,3 fused): h01 = 0.5*(|d23|+0.3)^2
                s01 = prep.tile([128, 32], F32, tag="s01", name=_r + "s01")
                s013 = s01.rearrange("p (t k) -> p t k", k=2)
                nc.scalar.activation(s013[:, :, :], d8v[:, :, 2:4], AF.Abs)
                nc.vector.tensor_scalar_add(s01[:], s01[:], 0.3)
                h01 = prep.tile([128, 32], F32, tag="h01", name=_r + "h01")
                nc.scalar.activation(
                    h01[:], s01[:], AF.Square, scale=float(np.sqrt(0.5))
                )
                h013 = h01.rearrange("p (t k) -> p t k", k=2)

                sum5 = t16("sum5")  # 0.5*(s0^2+s1^2)
                nc.vector.tensor_tensor(sum5[:], h013[:, :, 0], h013[:, :, 1], ADD)
                dif5 = t16("dif5")  # 0.5*(s0^2-s1^2)
                nc.vector.tensor_tensor(dif5[:], h013[:, :, 0], h013[:, :, 1], SUB)

                # covariance entries, packed k-major so the conic divide
                # is one broadcast multiply: cov3 rows (cov_c, cov_a, cov_b)
                cov3 = prep.tile([128, 48], F32, tag="cov3")
                cov3v = cov3.rearrange("p (k t) -> p k t", t=16)
                dc = t16("dc")
                nc.vector.tensor_tensor(dc[:], dif5[:], c2t[:], MUL)
                nc.vector.tensor_tensor(cov3v[:, 1, :], sum5[:], dc[:], ADD)
                nc.vector.tensor_tensor(cov3v[:, 0, :], sum5[:], dc[:], SUB)
                nc.vector.tensor_tensor(cov3v[:, 2, :], dif5[:], s2t[:], MUL)

                det = t16("det")
                nc.vector.tensor_tensor(det[:], cov3v[:, 1, :], cov3v[:, 0, :], MUL)
                bb = t16("bb")
                nc.scalar.activation(bb[:], cov3v[:, 2, :], AF.Square)
                nc.vector.tensor_tensor(det[:], det[:], bb[:], SUB)

                # conic: (ca, cc, cbn) = (cov_c, cov_a, cov_b) / det
                inv = t16("inv")
                nc.vector.reciprocal(inv[:], det[:])
                con3 = prep.tile([128, 48], F32, tag="con3")
                con3v = con3.rearrange("p (k t) -> p k t", t=16)
                inv_b = inv[:].unsqueeze(1).broadcast_to([128, 3, 16])
                nc.vector.tensor_tensor(con3v[:, :, :], cov3v[:, :, :], inv_b, MUL)
                con3t = con3.rearrange("p (k t) -> p t k", t=16)
                ca = con3v[:, 0, :]
                cc = con3v[:, 1, :]
                cbn = con3v[:, 2, :]

                # ---- F rows (negated for exp), written into Fc [128, 96] ----
                Fc = prep.tile([128, 96], F32, tag="Fc")
                Fv = Fc.rearrange("p (t k) -> p t k", k=6)

                exyq = prep.tile([128, 32], F32, tag="exyq", name=_r + "exyq")
                nc.scalar.activation(exyq[:], exy[:], AF.Square)
                exyq3 = exyq.rearrange("p (t k) -> p t k", k=2)
                exq = exyq3[:, :, 0]
                eyq = exyq3[:, :, 1]
                ve = nc.gpsimd if prep_gpsimd else nc.vector
                exey = t16("exey")
                ve.tensor_tensor(exey[:], ex, ey, MUL)

                # f0 = -0.5*ca*exq - 0.5*cc*eyq + cbn*exey
                t_a = t16("t_a")
                ve.tensor_tensor(t_a[:], ca, exq, MUL)
                t_b = t16("t_b")
                ve.tensor_tensor(t_b[:], cc, eyq, MUL)
                ve.tensor_tensor(t_a[:], t_a[:], t_b[:], ADD)
                ve.tensor_scalar_mul(t_a[:], t_a[:], -0.5)
                t_c = t16("t_c")
                ve.tensor_tensor(t_c[:], cbn, exey[:], MUL)
                ve.tensor_tensor(Fv[:, :, 0], t_a[:], t_c[:], ADD)

                # f_x = ca*ex - cbn*ey ; f_y = cc*ey - cbn*ex
                t_d = t16("t_d")
                ve.tensor_tensor(t_d[:], ca, ex, MUL)
                t_e = t16("t_e")
                ve.tensor_tensor(t_e[:], cbn, ey, MUL)
                ve.tensor_tensor(Fv[:, :, 1], t_d[:], t_e[:], SUB)
                t_f = t16("t_f")
                ve.tensor_tensor(t_f[:], cc, ey, MUL)
                t_g = t16("t_g")
                ve.tensor_tensor(t_g[:], cbn, ex, MUL)
                ve.tensor_tensor(Fv[:, :, 2], t_f[:], t_g[:], SUB)

                # f_x2 = -0.5*ca ; f_y2 = -0.5*cc ; f_xy = +cbn
                nc.vector.tensor_scalar_mul(Fv[:, :, 3:5], con3t[:, :, 0:2], -0.5)
                nc.vector.tensor_scalar_mul(Fv[:, :, 5], con3t[:, :, 2], 1.0)

                # ---- split F into fp16 hi/mid/lo, [128, 16*32] (stride 32
                # so one full [128,128] transpose per image lands each tile
                # at the 32-row group the sigma matmul reads) ----
                fall = prep.tile([128, NT * 32], F16, tag="fall")
                nc.vector.memset(fall[:], 0.0)
                fv = fall.rearrange("p (t s) -> p t s", s=32)
                Fc6 = Fc.rearrange("p (t k) -> p t k", k=6)
                nc.vector.tensor_copy(fv[:, :, 0:6], Fc6[:, :, :])
                r1 = prep.tile([128, 96], F32, tag="r1")
                r16 = r1.rearrange("p (t k) -> p t k", k=6)
                nc.vector.tensor_tensor(
                    r16[:, :, :], Fc6[:, :, :], fv[:, :, 0:6], SUB
                )
                nc.vector.tensor_copy(fv[:, :, 6:12], r16[:, :, :])
                r2 = prep.tile([128, 96], F32, tag="r2")
                r26 = r2.rearrange("p (t k) -> p t k", k=6)
                nc.vector.tensor_tensor(
                    r26[:, :, :], r16[:, :, :], fv[:, :, 6:12], SUB
                )
                nc.vector.tensor_copy(fv[:, :, 12:18], r26[:, :, :])

                # ---- per-image transpose: [128, 128] -> psum -> f4 ----
                with tc.tile_pool(name=_r + "prepps", bufs=2, space="PSUM") as prepps:
                    for img in range(B_CORE):
                        tp = prepps.tile(
                            [128, 128], F16, tag="tp", name=f"{_r}tp{img}"
                        )
                        nc.tensor.transpose(
                            tp[:], fall[:, img * 128 : (img + 1) * 128], idt[:]
                        )
                        nc.vector.tensor_copy(
                            f4[:, img * 128 : (img + 1) * 128], tp[:]
                        )

                # ---- colors * opacity -> c2 [128, 16*3] fp16 ----
                opac_b = opac[:].unsqueeze(1).broadcast_to([128, 4, 4])
                cP = prep.tile([128, 48], F32, tag="cP")
                cP4 = cP.rearrange("p (i n k) -> p i n k", n=4, k=3)
                d84 = d8.rearrange("p (i n k) -> p i n k", n=4, k=8)
                for k in range(3):
                    nc.vector.tensor_tensor(
                        cP4[:, :, :, k], d84[:, :, :, 5 + k], opac_b, MUL
                    )
                nc.vector.tensor_copy(c2[:], cP[:])

            if not skip_prep:
                _do_prep()

            # ---- main loop ----
            # alpha layout: seg-major [128, seg*2048 + nt*512 + q] so the
            # Exp write is fully contiguous and the blend rhs slices are
            # contiguous too (blend chunk ch == pixel segment seg).
            def al_exp_view(al, seg):
                if seg_major:
                    return al[:, seg * 2048 : (seg + 1) * 2048]
                return al.rearrange("p (t q) -> p t q", q=HW)[
                    :, :, seg * 512 : (seg + 1) * 512
                ]

            def al_blend_view(al, ch, nt):
                if seg_major:
                    off = ch * 2048 + nt * 512
                else:
                    off = nt * HW + ch * 512
                return al[:, off : off + 512]

            n_grp = sig_cols // 512  # row groups covered per PSUM tile
            with (
                tc.tile_pool(name=_r + "sigps", bufs=sig_bufs, space="PSUM") as sigps,
                tc.tile_pool(name=_r + "blps", bufs=blend_bufs, space="PSUM") as blps,
            ):
                als = {}

                def emit_sigma(img, seg):
                    al = als[img]
                    for h in range(4 // n_grp):
                        sps = sigps.tile(
                            [128, sig_cols], F32, tag="sig",
                            name=f"{_r}sig{img}_{seg}_{h}"
                        )
                        if not skip_sigma:
                            nts = [0] if min_sigma else range(n_grp)
                            for j in nts:
                                nt = h * n_grp + j
                                nc.tensor.matmul(
                                    sps[:, j * 512 : (j + 1) * 512],
                                    f4[32 * nt : 32 * nt + 18,
                                       img * 128 : (img + 1) * 128],
                                    g4[32 * nt : 32 * nt + 18,
                                       seg * 512 : (seg + 1) * 512],
                                    start=True,
                                    stop=True,
                                    tile_position=(32 * nt, 0),
                                )
                        if not skip_exp:
                            step = sig_cols // split_exp
                            base = h * sig_cols
                            for q in range(0, sig_cols, step):
                                nc.scalar.activation(
                                    al_exp_view(als[img], seg)[
                                        :, base + q : base + q + step
                                    ],
                                    sps[:, q : q + step],
                                    AF.Exp,
                                )

                def emit_blend(img, ch):
                    """blend_groups chunks starting at ch, packed into the
                    32-col groups of the PE array (one PSUM bank each, so
                    the accumulation groups stay bank-disjoint) and run
                    concurrently; a single VectorE copy + per-chunk DMA
                    evacuates them."""
                    al = als[img]
                    G = blend_groups
                    bps = blps.tile(
                        [128, G * 512] if G > 1 else [3, 512], F32,
                        tag="bl", name=f"{_r}bl{img}_{ch}"
                    )
                    for nt in range(4):
                        for g in range(G):
                            t = img * 4 + nt
                            rhs = (
                                g4[:, (ch + g) * 512 : (ch + g) * 512 + 512]
                                if skip_exp
                                else al_blend_view(al, ch + g, nt)
                            )
                            out = (
                                bps[32 * g : 32 * g + 3,
                                    g * 512 : (g + 1) * 512]
                                if G > 1 else bps[:]
                            )
                            nc.tensor.matmul(
                                out,
                                c2[:, t * 3 : t * 3 + 3],
                                rhs,
                                start=(nt == 0),
                                stop=(nt == 3),
                                tile_position=(0, 32 * g) if G > 1 else None,
                            )
                    np_out = 32 * (G - 1) + 3 if G > 1 else 3
                    ot = outp.tile(
                        [np_out, G * 512] if G > 1 else [3, 512], F32,
                        tag="ot", name=f"{_r}ot{img}_{ch}"
                    )
                    nc.vector.tensor_copy(ot[:], bps[0:np_out, :])
                    for g in range(G):
                        cg = ch + g
                        src = (
                            ot[32 * g : 32 * g + 3, g * 512 : (g + 1) * 512]
                            if G > 1 else ot[:]
                        )
                        nc.sync.dma_start(
                            d_img[img, :, cg * 8 : (cg + 1) * 8, :].rearrange(
                                "c h w -> c (h w)"
                            ),
                            src,
                        )

                # software pipeline: interleave img i's sigma/exp segments
                # with the blend chunk lying 8*blend_lag + blend_skew
                # chunk-slots behind, so each blend's alpha dependency
                # (exp of the same-index segment, seg-major layout) is
                # already satisfied when the in-order PE reaches it.
                # blend_lag=0 reproduces the unpipelined layout (all
                # blends right after the img's own segments).
                # blend_lag defers each image's blend block by that many
                # images (image granularity — blend and sigma matmul
                # blocks stay contiguous to avoid PE weight-array churn).
                for img in range(B_CORE + blend_lag):
                    if img < B_CORE:
                        als[img] = alphap.tile(
                            [128, 4 * HW], F16, tag="al", name=f"{_r}al{img}"
                        )
                        for seg in range(8):
                            emit_sigma(img, seg)
                    bimg = img - blend_lag
                    if bimg >= 0 and not skip_blend:
                        for ch in range(0, 8, blend_groups):
                            emit_blend(bimg, ch)

    nc.compile()
    return nc


_NC_CACHE = None


def _get_program():
    global _NC_CACHE
    if _NC_CACHE is None:
        _NC_CACHE = build_program()
    return _NC_CACHE


def make_in_maps(data, opacity):
    data = np.ascontiguousarray(np.asarray(data, dtype=np.float32))
    opacity = np.ascontiguousarray(np.asarray(opacity, dtype=np.float32))
    G4, ident = host_constants()
    in_maps = []
    op_pt = np.ascontiguousarray(opacity.reshape(4, 128).T)  # [p, ntile]
    for c in range(N_CORES):
        dc = data[c * B_CORE : (c + 1) * B_CORE].reshape(NG, 8)
        # device layout d8[p, t*8+k] = data[t*128+p, k]
        d8 = np.ascontiguousarray(
            dc.reshape(NT, 128, 8).transpose(1, 0, 2).reshape(128, 128)
        )
        in_maps.append(
            {"data": d8, "opacity": op_pt, "gconst": G4, "ident": ident}
        )
    return in_maps


class _Executor:
    """One-time jit of the sharded bass_exec program + device-resident
    constants; warm kernel() calls only upload data/opacity and download
    the image, avoiding the per-call re-trace/re-compile/NEFF-reload that
    run_bass_kernel_spmd pays."""

    def __init__(self):
        import jax
        from jax.sharding import Mesh, NamedSharding, PartitionSpec
        from jax.experimental.shard_map import shard_map
        from concourse import bass2jax

        bass2jax.install_neuronx_cc_hook()
        nc = _get_program()
        self.jax = jax
        partition_name = (
            nc.partition_id_tensor.name if nc.partition_id_tensor else None
        )
        in_names, out_names, out_avals, zero_outs = [], [], [], []
        for alloc in nc.m.functions[0].allocations:
            if not isinstance(alloc, mybir.MemoryLocationSet):
                continue
            name = alloc.memorylocations[0].name
            if alloc.kind == "ExternalInput":
                if name != partition_name:
                    in_names.append(name)
            elif alloc.kind == "ExternalOutput":
                out_names.append(name)
                shape = tuple(alloc.tensor_shape)
                dtype = mybir.dt.np(alloc.dtype)
                out_avals.append(jax.core.ShapedArray(shape, dtype))
                zero_outs.append(np.zeros(shape, dtype))
        n_params = len(in_names)
        n_outs = len(out_avals)
        all_in_names = list(in_names) + out_names
        if partition_name is not None:
            all_in_names.append(partition_name)

        def _body(*args):
            operands = list(args)
            if partition_name is not None:
                operands.append(bass2jax.partition_id_tensor())
            outs = bass2jax._bass_exec_p.bind(
                *operands,
                out_avals=tuple(out_avals),
                in_names=tuple(all_in_names),
                out_names=tuple(out_names),
                lowering_input_output_aliases=(),
                sim_require_finite=True,
                sim_require_nnan=True,
                nc=nc,
            )
            return tuple(outs)

        devices = jax.devices()[:N_CORES]
        mesh = Mesh(np.asarray(devices), ("core",))
        in_specs = (PartitionSpec("core"),) * (n_params + n_outs)
        out_specs = (PartitionSpec("core"),) * n_outs
        donate = tuple(range(n_params, n_params + n_outs))
        self.sharded = jax.jit(
            shard_map(
                _body,
                mesh=mesh,
                in_specs=in_specs,
                out_specs=out_specs,
                check_rep=False,
            ),
            donate_argnums=donate,
            keep_unused=True,
        )
        self.in_names = in_names
        self.out_names = out_names
        self.out_avals = out_avals
        sh = NamedSharding(mesh, PartitionSpec("core"))
        self.sharding = sh
        # constants resident on device, sharded by core
        G4, ident = host_constants()
        self.const_dev = {
            "gconst": jax.device_put(
                np.concatenate([G4] * N_CORES, axis=0), sh
            ),
            "ident": jax.device_put(
                np.concatenate([ident] * N_CORES, axis=0), sh
            ),
        }
        # donated output buffers: zeros on first call, recycled afterwards
        # (the kernel writes every output element)
        self.out_bufs = [
            jax.device_put(
                np.zeros((N_CORES * z.shape[0], *z.shape[1:]), z.dtype), sh
            )
            for z in zero_outs
        ]

    def run(self, per_core_inputs):
        """per_core_inputs: dict name -> list of per-core np arrays (for
        non-constant inputs)."""
        args = []
        for name in self.in_names:
            if name in self.const_dev:
                args.append(self.const_dev[name])
            else:
                args.append(np.concatenate(per_core_inputs[name], axis=0))
        outs = self.sharded(*args, *self.out_bufs)
        self.out_bufs = list(outs)
        return outs


_EXEC_CACHE = None


def _get_executor():
    global _EXEC_CACHE
    if _EXEC_CACHE is None:
        _EXEC_CACHE = _Executor()
    return _EXEC_CACHE


def kernel(data, opacity):
    ex = _get_executor()
    data = np.ascontiguousarray(np.asarray(data, dtype=np.float32))
    opacity = np.ascontiguousarray(np.asarray(opacity, dtype=np.float32))
    op_pt = np.ascontiguousarray(opacity.reshape(4, 128).T)
    d8s = [
        np.ascontiguousarray(
            data[c * B_CORE : (c + 1) * B_CORE]
            .reshape(NT, 128, 8)
            .transpose(1, 0, 2)
            .reshape(128, 128)
        )
        for c in range(N_CORES)
    ]
    outs = ex.run({"data": d8s, "opacity": [op_pt] * N_CORES})
    i_img = ex.out_names.index("img")
    img = np.asarray(outs[i_img]).reshape(N_CORES, B_CORE, 3, 64, 64)
    return img.reshape(B, 3, 64, 64).astype(np.float32)


# revision 36
# speedup vs baseline: 1.4643x; 1.0044x over previous
"""Trainium2 Bass kernel for the GaussianRenderer problem.

Contract: kernel(data, opacity) -> img
  data:    (32, 512, 8) float32
  opacity: (512, 1)     float32
  returns  (32, 3, 64, 64) float32

Sharding: data-parallel over batch B=32 across 8 NeuronCores (4 images
per core); no collectives.

Per-core algorithm (all compute on device):
  sigma[n, p] is a rank-6 bilinear form: sigma = F[n, :6] @ G[:6, p]
  where G rows are the pixel-coordinate monomials [1, x, y, x^2, y^2, xy]
  with x, y integer in [-32, 31] (exactly representable in fp16). F is
  derived per gaussian on-device; all transcendentals use only the
  exp_and_others ACT table set (tanh/abs/exp) -- the reference's
  sigmoid+sin are replaced by the identity sigmoid(x)-0.5 = tanh(x/2)/2
  plus Taylor polynomials for sin/cos(pi*v) and two double-angle steps
  on VectorE, so no ACT table reloads occur in steady state.

  F is split into fp16 hi/mid/lo parts and stacked K=18 so a single fp16
  TensorE matmul yields fp32-accurate (negated) sigma. The 16 gaussian
  tiles (4 images x 4 tiles of 128) are packed 4-per-image into the four
  32-row groups of the PE array (tile_position row tiling), so the four
  [18x128]@[18x512] sigma matmuls of one image run concurrently and fill
  four PSUM banks [128, 2048]; one ScalarE Exp instruction per 512-pixel
  segment evacuates all four banks to fp16 alpha in SBUF. Blending is a
  second TensorE matmul contracting the 128-gaussian partition dim with
  opacity-scaled fp16 colors (M=3), accumulated over the 4 gaussian
  tiles in PSUM, evacuated by a single VectorE copy, then DMA'd out.
"""

import numpy as np

import concourse.bacc as bacc
import concourse.mybir as mybir
import concourse.tile as tile
from concourse import bass_utils
from concourse._compat import get_trn_type
from concourse.alu_op_type import AluOpType

F32 = mybir.dt.float32
F16 = mybir.dt.float16
AF = mybir.ActivationFunctionType
MUL = AluOpType.mult
ADD = AluOpType.add
SUB = AluOpType.subtract

N_CORES = 8
B = 32
B_CORE = B // N_CORES  # 4 images per core
N = 512                # gaussians
NG = B_CORE * N        # gaussians handled per core
NT = 16                # gaussian tiles of 128 per core (4 img * 4 ntiles)
HW = 4096              # pixels per image (64 x 64)
PI = float(np.pi)


def host_constants():
    """G4 [128, 4096] fp16: partition 32*g + j (g=0..3, j=0..17) holds
    monomial row j%6 of [1, x, y, x^2, y^2, xy] (hi/mid/lo K-stacking of
    3 copies of the 6 rows), with x, y integer in [-32, 31]; partitions
    j>=18 are zero.  Plus fp16 identity for the PE transpose."""
    xs = np.arange(64, dtype=np.float64) - 32.0
    Xg, Yg = np.meshgrid(xs, xs)  # [h, w]; row-major pixels p = h*64 + w
    G = np.stack(
        [np.ones_like(Xg), Xg, Yg, Xg * Xg, Yg * Yg, Xg * Yg], 0
    ).reshape(6, HW)
    G18 = np.concatenate([G, G, G], 0)  # [18, HW]
    G4 = np.zeros((128, HW), np.float16)
    for g in range(4):
        G4[32 * g : 32 * g + 18] = G18
    ident = np.eye(128, dtype=np.float16)
    return G4, ident


def build_program(reps=1, loop=0, skip_prep=False, skip_sigma=False,
                  skip_exp=False, skip_blend=False, seg_major=True,
                  split_exp=1, sig_bufs=2, sig_cols=1024, blend_bufs=1,
                  blend_lag=1, blend_skew=0, blend_groups=4, min_sigma=False,
                  prep_gpsimd=False):
    import contextlib

    nc = bacc.Bacc(get_trn_type() or "TRN2", target_bir_lowering=False, debug=False)
    # host pre-permutes into the on-chip layouts so these DMAs are contiguous
    d_data = nc.dram_tensor("data", (128, 128), F32, kind="ExternalInput")
    d_opac = nc.dram_tensor("opacity", (128, 4), F32, kind="ExternalInput")
    d_g4 = nc.dram_tensor("gconst", (128, HW), F16, kind="ExternalInput")
    d_id = nc.dram_tensor("ident", (128, 128), F16, kind="ExternalInput")
    d_img = nc.dram_tensor("img", (B_CORE, 3, 64, 64), F32, kind="ExternalOutput")

    with tile.TileContext(nc) as tc:
      _loop_kw = dict(
          hint_engines=(
              mybir.EngineType.PE,
              mybir.EngineType.Activation,
              mybir.EngineType.DVE,
              mybir.EngineType.SP,
              mybir.EngineType.Pool,
          )
      )
      with tc.For_i(0, loop, 1, **_loop_kw) if loop else contextlib.nullcontext():
       for rep in range(reps):
        _r = f"r{rep}_" if reps > 1 else ""
        with (
            tc.tile_pool(name=_r + "const", bufs=1) as constp,
            tc.tile_pool(name=_r + "prep", bufs=1) as prep,
            tc.tile_pool(name=_r + "alpha", bufs=blend_lag + 2) as alphap,
            tc.tile_pool(name=_r + "outp", bufs=4) as outp,
        ):
            # ---- constants + inputs to SBUF ----
            g4 = constp.tile([128, HW], F16, tag="g4")
            nc.sync.dma_start(g4[:], d_g4[:])
            idt = constp.tile([128, 128], F16, tag="idt")
            nc.sync.dma_start(idt[:], d_id[:])
            d8 = constp.tile([128, 128], F32, tag="d8")  # [p, t*8+k]
            nc.sync.dma_start(d8[:], d_data[:])
            opac = constp.tile([128, 4], F32, tag="opac")  # [p, ntile]
            nc.sync.dma_start(opac[:], d_opac[:])

            f4 = constp.tile([128, B_CORE * 128], F16, tag="f4")
            c2 = constp.tile([128, NT * 3], F16, tag="c2")

            d8v = d8.rearrange("p (t k) -> p t k", k=8)

            def field(k):  # [128, 16] strided view of input field k
                return d8v[:, :, k]

            def t16(tag):
                return prep.tile([128, 16], F32, tag=tag, name=_r + tag)

            def _do_prep():
                # ---- per-gaussian preprocessing ([128, 16] fp32 tiles) ----
                # theta = 2*pi*sigmoid(d4); only cos/sin(2*theta) are needed.
                # v = sigmoid(d4) - 0.5 = tanh(d4/2)/2, so with p = pi*v:
                # cos(2t) = cos(4*pi*v), sin(2t) = sin(4*pi*v), built from
                # Taylor cos/sin(p) (|p| < pi/2) + two double-angle steps.
                # Everything stays in the exp_and_others ACT table set.
                th = t16("th")
                nc.scalar.activation(th[:], field(4), AF.Tanh, scale=0.5)
                pv = t16("pv")
                nc.vector.tensor_scalar_mul(pv[:], th[:], PI / 2)
                wv = t16("wv")
                nc.scalar.activation(wv[:], pv[:], AF.Square)

                def horner(dst, coeffs, tail):
                    # dst = (((c0*w + c1)*w + c2)...)*w + tail
                    nc.vector.tensor_scalar(
                        dst[:], wv[:], coeffs[0], coeffs[1], MUL, ADD
                    )
                    for ck in coeffs[2:] + [tail]:
                        nc.vector.tensor_tensor(dst[:], dst[:], wv[:], MUL)
                        nc.vector.tensor_scalar_add(dst[:], dst[:], ck)

                cw = t16("cw")  # cos(p)
                horner(
                    cw,
                    [-1.0 / 3628800, 1.0 / 40320, -1.0 / 720, 1.0 / 24, -0.5],
                    1.0,
                )
                sw = t16("sw")  # sin(p) = p * poly(w)
                horner(sw, [1.0 / 362880, -1.0 / 5040, 1.0 / 120, -1.0 / 6], 1.0)
                nc.vector.tensor_tensor(sw[:], sw[:], pv[:], MUL)

                c1t = t16("c1t")  # cos(2p)
                nc.vector.tensor_tensor(c1t[:], sw[:], sw[:], MUL)
                nc.vector.tensor_scalar(c1t[:], c1t[:], -2.0, 1.0, MUL, ADD)
                s1t = t16("s1t")  # sin(2p)
                nc.vector.scalar_tensor_tensor(s1t[:], sw[:], 2.0, cw[:], MUL, MUL)
                c2t = t16("c2t")  # cos(4p) = cos(2*theta)
                nc.vector.tensor_tensor(c2t[:], s1t[:], s1t[:], MUL)
                nc.vector.tensor_scalar(c2t[:], c2t[:], -2.0, 1.0, MUL, ADD)
                s2t = t16("s2t")  # sin(4p) = sin(2*theta)
                nc.vector.scalar_tensor_tensor(s2t[:], s1t[:], 2.0, c1t[:], MUL, MUL)

                # centers (fields 0,1 fused): exy = 32*tanh(d01) - 0.5
                exy = prep.tile([128, 32], F32, tag="exy", name=_r + "exy")
                exy3 = exy.rearrange("p (t k) -> p t k", k=2)
                nc.scalar.activation(exy3[:, :, :], d8v[:, :, 0:2], AF.Tanh)
                nc.vector.tensor_scalar(exy[:], exy[:], 32.0, -0.5, MUL, ADD)
                ex = exy3[:, :, 0]
                ey = exy3[:, :, 1]

                # scales (fields 2,3 fused): h01 = 0.5*(|d23|+0.3)^2
                s01 = prep.tile([128, 32], F32, tag="s01", name=_r + "s01")
                s013 = s01.rearrange("p (t k) -> p t k", k=2)
                nc.scalar.activation(s013[:, :, :], d8v[:, :, 2:4], AF.Abs)
                nc.vector.tensor_scalar_add(s01[:], s01[:], 0.3)
                h01 = prep.tile([128, 32], F32, tag="h01", name=_r + "h01")
                nc.scalar.activation(
                    h01[:], s01[:], AF.Square, scale=float(np.sqrt(0.5))
                )
                h013 = h01.rearrange("p (t k) -> p t k", k=2)

                sum5 = t16("sum5")  # 0.5*(s0^2+s1^2)
                nc.vector.tensor_tensor(sum5[:], h013[:, :, 0], h013[:, :, 1], ADD)
                dif5 = t16("dif5")  # 0.5*(s0^2-s1^2)
                nc.vector.tensor_tensor(dif5[:], h013[:, :, 0], h013[:, :, 1], SUB)

                # covariance entries, packed k-major so the conic divide
                # is one broadcast multiply: cov3 rows (cov_c, cov_a, cov_b)
                cov3 = prep.tile([128, 48], F32, tag="cov3")
                cov3v = cov3.rearrange("p (k t) -> p k t", t=16)
                dc = t16("dc")
                nc.vector.tensor_tensor(dc[:], dif5[:], c2t[:], MUL)
                nc.vector.tensor_tensor(cov3v[:, 1, :], sum5[:], dc[:], ADD)
                nc.vector.tensor_tensor(cov3v[:, 0, :], sum5[:], dc[:], SUB)
                nc.vector.tensor_tensor(cov3v[:, 2, :], dif5[:], s2t[:], MUL)

                det = t16("det")
                nc.vector.tensor_tensor(det[:], cov3v[:, 1, :], cov3v[:, 0, :], MUL)
                bb = t16("bb")
                nc.scalar.activation(bb[:], cov3v[:, 2, :], AF.Square)
                nc.vector.tensor_tensor(det[:], det[:], bb[:], SUB)

                # conic: (ca, cc, cbn) = (cov_c, cov_a, cov_b) / det
                inv = t16("inv")
                nc.vector.reciprocal(inv[:], det[:])
                con3 = prep.tile([128, 48], F32, tag="con3")
                con3v = con3.rearrange("p (k t) -> p k t", t=16)
                inv_b = inv[:].unsqueeze(1).broadcast_to([128, 3, 16])
                nc.vector.tensor_tensor(con3v[:, :, :], cov3v[:, :, :], inv_b, MUL)
                con3t = con3.rearrange("p (k t) -> p t k", t=16)
                ca = con3v[:, 0, :]
                cc = con3v[:, 1, :]
                cbn = con3v[:, 2, :]

                # ---- F rows (negated for exp), written into Fc [128, 96] ----
                Fc = prep.tile([128, 96], F32, tag="Fc")
                Fv = Fc.rearrange("p (t k) -> p t k", k=6)

                exyq = prep.tile([128, 32], F32, tag="exyq", name=_r + "exyq")
                nc.scalar.activation(exyq[:], exy[:], AF.Square)
                exyq3 = exyq.rearrange("p (t k) -> p t k", k=2)
                exq = exyq3[:, :, 0]
                eyq = exyq3[:, :, 1]
                ve = nc.gpsimd if prep_gpsimd else nc.vector
                exey = t16("exey")
                ve.tensor_tensor(exey[:], ex, ey, MUL)

                # f0 = -0.5*ca*exq - 0.5*cc*eyq + cbn*exey
                t_a = t16("t_a")
                ve.tensor_tensor(t_a[:], ca, exq, MUL)
                t_b = t16("t_b")
                ve.tensor_tensor(t_b[:], cc, eyq, MUL)
                ve.tensor_tensor(t_a[:], t_a[:], t_b[:], ADD)
                ve.tensor_scalar_mul(t_a[:], t_a[:], -0.5)
                t_c = t16("t_c")
                ve.tensor_tensor(t_c[:], cbn, exey[:], MUL)
                ve.tensor_tensor(Fv[:, :, 0], t_a[:], t_c[:], ADD)

                # f_x = ca*ex - cbn*ey ; f_y = cc*ey - cbn*ex
                t_d = t16("t_d")
                ve.tensor_tensor(t_d[:], ca, ex, MUL)
                t_e = t16("t_e")
                ve.tensor_tensor(t_e[:], cbn, ey, MUL)
                ve.tensor_tensor(Fv[:, :, 1], t_d[:], t_e[:], SUB)
                t_f = t16("t_f")
                ve.tensor_tensor(t_f[:], cc, ey, MUL)
                t_g = t16("t_g")
                ve.tensor_tensor(t_g[:], cbn, ex, MUL)
                ve.tensor_tensor(Fv[:, :, 2], t_f[:], t_g[:], SUB)

                # f_x2 = -0.5*ca ; f_y2 = -0.5*cc ; f_xy = +cbn
                nc.vector.tensor_scalar_mul(Fv[:, :, 3:5], con3t[:, :, 0:2], -0.5)
                nc.vector.tensor_scalar_mul(Fv[:, :, 5], con3t[:, :, 2], 1.0)

                # ---- split F into fp16 hi/mid/lo, [128, 16*32] (stride 32
                # so one full [128,128] transpose per image lands each tile
                # at the 32-row group the sigma matmul reads) ----
                fall = prep.tile([128, NT * 32], F16, tag="fall")
                nc.vector.memset(fall[:], 0.0)
                fv = fall.rearrange("p (t s) -> p t s", s=32)
                Fc6 = Fc.rearrange("p (t k) -> p t k", k=6)
                nc.vector.tensor_copy(fv[:, :, 0:6], Fc6[:, :, :])
                r1 = prep.tile([128, 96], F32, tag="r1")
                r16 = r1.rearrange("p (t k) -> p t k", k=6)
                nc.vector.tensor_tensor(
                    r16[:, :, :], Fc6[:, :, :], fv[:, :, 0:6], SUB
                )
                nc.vector.tensor_copy(fv[:, :, 6:12], r16[:, :, :])
                r2 = prep.tile([128, 96], F32, tag="r2")
                r26 = r2.rearrange("p (t k) -> p t k", k=6)
                nc.vector.tensor_tensor(
                    r26[:, :, :], r16[:, :, :], fv[:, :, 6:12], SUB
                )
                nc.vector.tensor_copy(fv[:, :, 12:18], r26[:, :, :])

                # ---- per-image transpose: [128, 128] -> psum -> f4 ----
                with tc.tile_pool(name=_r + "prepps", bufs=2, space="PSUM") as prepps:
                    for img in range(B_CORE):
                        tp = prepps.tile(
                            [128, 128], F16, tag="tp", name=f"{_r}tp{img}"
                        )
                        nc.tensor.transpose(
                            tp[:], fall[:, img * 128 : (img + 1) * 128], idt[:]
                        )
                        nc.vector.tensor_copy(
                            f4[:, img * 128 : (img + 1) * 128], tp[:]
                        )

                # ---- colors * opacity -> c2 [128, 16*3] fp16 ----
                opac_b = opac[:].unsqueeze(1).broadcast_to([128, 4, 4])
                cP = prep.tile([128, 48], F32, tag="cP")
                cP4 = cP.rearrange("p (i n k) -> p i n k", n=4, k=3)
                d84 = d8.rearrange("p (i n k) -> p i n k", n=4, k=8)
                for k in range(3):
                    nc.vector.tensor_tensor(
                        cP4[:, :, :, k], d84[:, :, :, 5 + k], opac_b, MUL
                    )
                nc.vector.tensor_copy(c2[:], cP[:])

            if not skip_prep:
                _do_prep()

            # ---- main loop ----
            # alpha layout: seg-major [128, seg*2048 + nt*512 + q] so the
            # Exp write is fully contiguous and the blend rhs slices are
            # contiguous too (blend chunk ch == pixel segment seg).
            def al_exp_view(al, seg):
                if seg_major:
                    return al[:, seg * 2048 : (seg + 1) * 2048]
                return al.rearrange("p (t q) -> p t q", q=HW)[
                    :, :, seg * 512 : (seg + 1) * 512
                ]

            def al_blend_view(al, ch, nt):
                if seg_major:
                    off = ch * 2048 + nt * 512
                else:
                    off = nt * HW + ch * 512
                return al[:, off : off + 512]

            n_grp = sig_cols // 512  # row groups covered per PSUM tile
            with (
                tc.tile_pool(name=_r + "sigps", bufs=sig_bufs, space="PSUM") as sigps,
                tc.tile_pool(name=_r + "blps", bufs=blend_bufs, space="PSUM") as blps,
            ):
                als = {}

                def emit_sigma(img, seg):
                    al = als[img]
                    for h in range(4 // n_grp):
                        sps = sigps.tile(
                            [128, sig_cols], F32, tag="sig",
                            name=f"{_r}sig{img}_{seg}_{h}"
                        )
                        if not skip_sigma:
                            nts = [0] if min_sigma else range(n_grp)
                            for j in nts:
                                nt = h * n_grp + j
                                nc.tensor.matmul(
                                    sps[:, j * 512 : (j + 1) * 512],
                                    f4[32 * nt : 32 * nt + 18,
                                       img * 128 : (img + 1) * 128],
                                    g4[32 * nt : 32 * nt + 18,
                                       seg * 512 : (seg + 1) * 512],
                                    start=True,
                                    stop=True,
                                    tile_position=(32 * nt, 0),
                                )
                        if not skip_exp:
                            step = sig_cols // split_exp
                            base = h * sig_cols
                            for q in range(0, sig_cols, step):
                                nc.scalar.activation(
                                    al_exp_view(als[img], seg)[
                                        :, base + q : base + q + step
                                    ],
                                    sps[:, q : q + step],
                                    AF.Exp,
                                )

                def emit_blend(img, ch):
                    """blend_groups chunks starting at ch, packed into the
                    32-col groups of the PE array (one PSUM bank each, so
                    the accumulation groups stay bank-disjoint) and run
                    concurrently; a single VectorE copy + per-chunk DMA
                    evacuates them."""
                    al = als[img]
                    G = blend_groups
                    bps = blps.tile(
                        [128, G * 512] if G > 1 else [3, 512], F32,
                        tag="bl", name=f"{_r}bl{img}_{ch}"
                    )
                    for nt in range(4):
                        for g in range(G):
                            t = img * 4 + nt
                            rhs = (
                                g4[:, (ch + g) * 512 : (ch + g) * 512 + 512]
                                if skip_exp
                                else al_blend_view(al, ch + g, nt)
                            )
                            out = (
                                bps[32 * g : 32 * g + 3,
                                    g * 512 : (g + 1) * 512]
                                if G > 1 else bps[:]
                            )
                            nc.tensor.matmul(
                                out,
                                c2[:, t * 3 : t * 3 + 3],
                                rhs,
                                start=(nt == 0),
                                stop=(nt == 3),
                                tile_position=(0, 32 * g) if G > 1 else None,
                            )
                    # per-chunk combine: releases each bank's WAR for the
                    # next blend burst as soon as that chunk is evacuated,
                    # instead of serializing a whole-tile copy into PE's
                    # in-order stream.
                    for g in range(G):
                        cg = ch + g
                        ot = outp.tile(
                            [3, 512], F32, tag="ot", name=f"{_r}ot{img}_{cg}"
                        )
                        src = (
                            bps[32 * g : 32 * g + 3, g * 512 : (g + 1) * 512]
                            if G > 1 else bps[:]
                        )
                        nc.vector.tensor_copy(ot[:], src)
                        nc.sync.dma_start(
                            d_img[img, :, cg * 8 : (cg + 1) * 8, :].rearrange(
                                "c h w -> c (h w)"
                            ),
                            ot[:],
                        )

                # software pipeline: interleave img i's sigma/exp segments
                # with the blend chunk lying 8*blend_lag + blend_skew
                # chunk-slots behind, so each blend's alpha dependency
                # (exp of the same-index segment, seg-major layout) is
                # already satisfied when the in-order PE reaches it.
                # blend_lag=0 reproduces the unpipelined layout (all
                # blends right after the img's own segments).
                # blend_lag defers each image's blend block by that many
                # images (image granularity — blend and sigma matmul
                # blocks stay contiguous to avoid PE weight-array churn).
                for img in range(B_CORE + blend_lag):
                    if img < B_CORE:
                        als[img] = alphap.tile(
                            [128, 4 * HW], F16, tag="al", name=f"{_r}al{img}"
                        )
                        for seg in range(8):
                            emit_sigma(img, seg)
                    bimg = img - blend_lag
                    if bimg >= 0 and not skip_blend:
                        for ch in range(0, 8, blend_groups):
                            emit_blend(bimg, ch)

    nc.compile()
    return nc


_NC_CACHE = None


def _get_program():
    global _NC_CACHE
    if _NC_CACHE is None:
        _NC_CACHE = build_program()
    return _NC_CACHE


def make_in_maps(data, opacity):
    data = np.ascontiguousarray(np.asarray(data, dtype=np.float32))
    opacity = np.ascontiguousarray(np.asarray(opacity, dtype=np.float32))
    G4, ident = host_constants()
    in_maps = []
    op_pt = np.ascontiguousarray(opacity.reshape(4, 128).T)  # [p, ntile]
    for c in range(N_CORES):
        dc = data[c * B_CORE : (c + 1) * B_CORE].reshape(NG, 8)
        # device layout d8[p, t*8+k] = data[t*128+p, k]
        d8 = np.ascontiguousarray(
            dc.reshape(NT, 128, 8).transpose(1, 0, 2).reshape(128, 128)
        )
        in_maps.append(
            {"data": d8, "opacity": op_pt, "gconst": G4, "ident": ident}
        )
    return in_maps


class _Executor:
    """One-time jit of the sharded bass_exec program + device-resident
    constants; warm kernel() calls only upload data/opacity and download
    the image, avoiding the per-call re-trace/re-compile/NEFF-reload that
    run_bass_kernel_spmd pays."""

    def __init__(self):
        import jax
        from jax.sharding import Mesh, NamedSharding, PartitionSpec
        from jax.experimental.shard_map import shard_map
        from concourse import bass2jax

        bass2jax.install_neuronx_cc_hook()
        nc = _get_program()
        self.jax = jax
        partition_name = (
            nc.partition_id_tensor.name if nc.partition_id_tensor else None
        )
        in_names, out_names, out_avals, zero_outs = [], [], [], []
        for alloc in nc.m.functions[0].allocations:
            if not isinstance(alloc, mybir.MemoryLocationSet):
                continue
            name = alloc.memorylocations[0].name
            if alloc.kind == "ExternalInput":
                if name != partition_name:
                    in_names.append(name)
            elif alloc.kind == "ExternalOutput":
                out_names.append(name)
                shape = tuple(alloc.tensor_shape)
                dtype = mybir.dt.np(alloc.dtype)
                out_avals.append(jax.core.ShapedArray(shape, dtype))
                zero_outs.append(np.zeros(shape, dtype))
        n_params = len(in_names)
        n_outs = len(out_avals)
        all_in_names = list(in_names) + out_names
        if partition_name is not None:
            all_in_names.append(partition_name)

        def _body(*args):
            operands = list(args)
            if partition_name is not None:
                operands.append(bass2jax.partition_id_tensor())
            outs = bass2jax._bass_exec_p.bind(
                *operands,
                out_avals=tuple(out_avals),
                in_names=tuple(all_in_names),
                out_names=tuple(out_names),
                lowering_input_output_aliases=(),
                sim_require_finite=True,
                sim_require_nnan=True,
                nc=nc,
            )
            return tuple(outs)

        devices = jax.devices()[:N_CORES]
        mesh = Mesh(np.asarray(devices), ("core",))
        in_specs = (PartitionSpec("core"),) * (n_params + n_outs)
        out_specs = (PartitionSpec("core"),) * n_outs
        donate = tuple(range(n_params, n_params + n_outs))
        self.sharded = jax.jit(
            shard_map(
                _body,
                mesh=mesh,
                in_specs=in_specs,
                out_specs=out_specs,
                check_rep=False,
            ),
            donate_argnums=donate,
            keep_unused=True,
        )
        self.in_names = in_names
        self.out_names = out_names
        self.out_avals = out_avals
        sh = NamedSharding(mesh, PartitionSpec("core"))
        self.sharding = sh
        # constants resident on device, sharded by core
        G4, ident = host_constants()
        self.const_dev = {
            "gconst": jax.device_put(
                np.concatenate([G4] * N_CORES, axis=0), sh
            ),
            "ident": jax.device_put(
                np.concatenate([ident] * N_CORES, axis=0), sh
            ),
        }
        # donated output buffers: zeros on first call, recycled afterwards
        # (the kernel writes every output element)
        self.out_bufs = [
            jax.device_put(
                np.zeros((N_CORES * z.shape[0], *z.shape[1:]), z.dtype), sh
            )
            for z in zero_outs
        ]

    def run(self, per_core_inputs):
        """per_core_inputs: dict name -> list of per-core np arrays (for
        non-constant inputs)."""
        args = []
        for name in self.in_names:
            if name in self.const_dev:
                args.append(self.const_dev[name])
            else:
                args.append(np.concatenate(per_core_inputs[name], axis=0))
        outs = self.sharded(*args, *self.out_bufs)
        self.out_bufs = list(outs)
        return outs


_EXEC_CACHE = None


def _get_executor():
    global _EXEC_CACHE
    if _EXEC_CACHE is None:
        _EXEC_CACHE = _Executor()
    return _EXEC_CACHE


def kernel(data, opacity):
    ex = _get_executor()
    data = np.ascontiguousarray(np.asarray(data, dtype=np.float32))
    opacity = np.ascontiguousarray(np.asarray(opacity, dtype=np.float32))
    op_pt = np.ascontiguousarray(opacity.reshape(4, 128).T)
    d8s = [
        np.ascontiguousarray(
            data[c * B_CORE : (c + 1) * B_CORE]
            .reshape(NT, 128, 8)
            .transpose(1, 0, 2)
            .reshape(128, 128)
        )
        for c in range(N_CORES)
    ]
    outs = ex.run({"data": d8s, "opacity": [op_pt] * N_CORES})
    i_img = ex.out_names.index("img")
    img = np.asarray(outs[i_img]).reshape(N_CORES, B_CORE, 3, 64, 64)
    return img.reshape(B, 3, 64, 64).astype(np.float32)
